# revision 3
# baseline (speedup 1.0000x reference)
"""Trainium2 Bass kernel for nn_ComplexMixture.

Reference:
  output_real[b,n,m] = sum_s w[b,s] * (r[b,s,n]*r[b,s,m] + i[b,s,n]*i[b,s,m])
  output_imag[b,n,m] = sum_s w[b,s] * (i[b,s,n]*r[b,s,m] - r[b,s,n]*i[b,s,m])

Shapes: B=32, S=128, N=256. w is uniform [0,1) so sqrt(w) is real.

out_r is symmetric and out_i is antisymmetric, so the device only computes
  P = out_r + out_i
and the host recovers out_r = (P + P^T)/2, out_i = (P - P^T)/2.
The host pre-scales the inputs: Yr = sqrt(w)[:,None]*r, Yi = sqrt(w)[:,None]*i.
With U = Yr - Yi, V = Yr + Yi:
  P[n,m] = sum_s Yr[s,n]*U[s,m] + Yi[s,n]*V[s,m]
i.e. per 128-row output chunk c:  P_c = Yr_c.T @ U + Yi_c.T @ V  (PSUM accumulation).

All device data is fp16: halves DMA bytes vs fp32 and runs the PE at
1 cycle/row instead of 4 (PSUM accumulation stays fp32; end-to-end rel
err ~1e-3 vs the 2e-2 gate).

Data-parallel over B across 8 cores, 4 batches/core.

Per-core schedule (engine queues are FIFO; assignments below keep every
dependency chain's consumer on an otherwise-idle queue):
  sync:   dma-in b0, dma-in b2+b3, dma-out b0, b2, b3-half1
  scalar: dma-in b1, copy b1, dma-out b1, copy b3-half2, dma-out b3-half2
  vector: U=Yr-Yi for b0..b3, copy b0, b2, b3-half1 (PSUM fp32 -> fp16)
  gpsimd: junk memset (PE warmup data), V=Yr+Yi for b0..b3  (no SWDGE
          DMAs at all -- avoids the ~2.3us SWDGE dge_drain at exit)
  tensor: N_WARMUP f32r warmup matmuls on junk (ramps the PE p-state to
          full clock during the input-DMA dead time), then 16 fp16
          matmuls (4 per batch).
"""

import os

import numpy as np

import concourse.bass as bass
import concourse.mybir as mybir
import concourse.tile as tile
from concourse import bacc
from concourse.bass_utils import run_bass_kernel_spmd

B, S, N = 32, 128, 256
NCORES = 8
BPC = B // NCORES  # batches per core
XCOL = 2 * N * BPC  # 2048 fp16 columns

F32 = mybir.dt.float32
F16 = mybir.dt.float16
N_WARMUP = int(os.environ.get("CM_WARMUP", "12"))

LAST_RESULTS = None  # stashed BassKernelResults for test harness introspection


def build_nc() -> bass.Bass:
    nc = bacc.Bacc(num_swdge_queues=1)
    xin = nc.dram_tensor("xpack", [S, XCOL], F16, kind="ExternalInput")
    out = nc.dram_tensor("out_all", [BPC, 128, 2, N], F16, kind="ExternalOutput")

    with tile.TileContext(nc) as tc:
        with (
            tc.tile_pool(name="io", bufs=1) as io_pool,
            tc.tile_pool(name="yp", bufs=BPC) as y_pool,
            tc.tile_pool(name="op", bufs=BPC) as out_pool,
            tc.tile_pool(name="ps", bufs=BPC, space="PSUM") as ps_pool,
            tc.tile_pool(name="wu", bufs=1, space="PSUM") as wu_pool,
        ):
            # PE warmup scratch: small memset so the first warmup matmul can
            # start as early as possible after the tile-entry barrier.
            if N_WARMUP:
                junk = io_pool.tile([S, 128], F32, tag="junk", name="junk")
                nc.gpsimd.memset(junk, 1.0)
                junk_r = junk.bitcast(mybir.dt.float32r)
                wups = wu_pool.tile([128, 128], F32, tag="wu", name="wups")
                for k in range(N_WARMUP):
                    nc.tensor.matmul(
                        wups, lhsT=junk_r, rhs=junk_r,
                        start=True, stop=True, skip_group_check=True,
                    )

            X_all = io_pool.tile([S, XCOL], F16, tag="X", name="X_all")
            BW = 2 * N  # columns per batch
            # All input on the two HWDGE rings: b0 then b2+b3 on SP (sync),
            # b1 on ACT (scalar).
            nc.sync.dma_start(out=X_all[:, 0:BW], in_=xin[:, 0:BW])
            nc.scalar.dma_start(out=X_all[:, BW : 2 * BW], in_=xin[:, BW : 2 * BW])
            nc.sync.dma_start(out=X_all[:, 2 * BW : 4 * BW], in_=xin[:, 2 * BW : 4 * BW])

            def xparts(b):
                X = X_all[:, b * BW : (b + 1) * BW]
                return X[:, 0:N], X[:, N : 2 * N]

            # Phase 1: UV transforms. U on vector, V on gpsimd (parallel).
            UVs = []
            for b in range(BPC):
                Yr, Yi = xparts(b)
                UV = y_pool.tile([S, 2 * N], F16, tag="UV", name=f"UV{b}")
                nc.vector.tensor_sub(UV[:, 0:N], Yr, Yi)
                nc.gpsimd.tensor_add(UV[:, N : 2 * N], Yr, Yi)
                UVs.append(UV)

            # Phase 2: matmuls (tensor engine).
            pss = []
            for b in range(BPC):
                Yr, Yi = xparts(b)
                UV = UVs[b]
                ps = ps_pool.tile([128, 2 * N], F32, tag="ps", name=f"ps{b}")
                for c in range(2):
                    csl = slice(c * 128, c * 128 + 128)
                    osl = slice(c * N, (c + 1) * N)
                    nc.tensor.matmul(ps[:, osl], lhsT=Yr[:, csl], rhs=UV[:, 0:N], start=True, stop=False)
                    nc.tensor.matmul(ps[:, osl], lhsT=Yi[:, csl], rhs=UV[:, N : 2 * N], start=False, stop=True)
                pss.append(ps)

            # Phase 3: PSUM -> SBUF fp16 downcast copies + output DMAs.
            Os = [out_pool.tile([128, 2 * N], F16, tag="O", name=f"O{b}") for b in range(BPC)]
            dsts = [out[b].rearrange("p c m -> p (c m)") for b in range(BPC)]

            # b0: vector copy, sync kick
            nc.vector.tensor_copy(Os[0], pss[0])
            nc.sync.dma_start(out=dsts[0], in_=Os[0])
            # b1: scalar copy, scalar kick
            nc.scalar.copy(out=Os[1], in_=pss[1])
            nc.scalar.dma_start(out=dsts[1], in_=Os[1])
            # b2: vector copy, sync kick
            nc.vector.tensor_copy(Os[2], pss[2])
            nc.sync.dma_start(out=dsts[2], in_=Os[2])
            # b3: split halves across vector+scalar, kicks on sync+scalar
            nc.vector.tensor_copy(Os[3][:, 0:N], pss[3][:, 0:N])
            nc.sync.dma_start(out=out[3][:, 0, :], in_=Os[3][:, 0:N])
            nc.scalar.copy(out=Os[3][:, N : 2 * N], in_=pss[3][:, N : 2 * N])
            nc.scalar.dma_start(out=out[3][:, 1, :], in_=Os[3][:, N : 2 * N])
    nc.compile()
    return nc


def kernel(**inputs: np.ndarray):
    global LAST_RESULTS
    r = np.asarray(inputs["input_real"], dtype=np.float32)
    i = np.asarray(inputs["input_imag"], dtype=np.float32)
    w = np.ascontiguousarray(np.asarray(inputs["weight"], dtype=np.float32))
    assert r.shape == (B, S, N) and i.shape == (B, S, N) and w.shape == (B, S)

    # [B, 2, S, N] -> per-core [S, (b t n)] batch-major blocks, fp16
    sws = np.sqrt(w)  # [B, S]
    xin = (np.stack([r, i], axis=1) * sws[:, None, :, None]).astype(np.float16)

    in_maps = []
    for c in range(NCORES):
        sl = slice(c * BPC, (c + 1) * BPC)
        xpack = np.transpose(xin[sl], (2, 0, 1, 3)).reshape(S, XCOL)
        in_maps.append({"xpack": np.ascontiguousarray(xpack)})

    nc = build_nc()
    res = run_bass_kernel_spmd(nc, in_maps, core_ids=list(range(NCORES)))
    LAST_RESULTS = res

    out_all = np.concatenate(
        [np.asarray(res.results[c]["out_all"]) for c in range(NCORES)], axis=0
    )  # [B, 128, 2, N] fp16; P[b, c*128+p, m] = out_all[b, p, c, m]
    P = np.transpose(out_all.astype(np.float32), (0, 2, 1, 3)).reshape(B, N, N)
    Pt = np.transpose(P, (0, 2, 1))
    out_r = (P + Pt) * np.float32(0.5)
    out_i = (P - Pt) * np.float32(0.5)
    return (np.ascontiguousarray(out_r), np.ascontiguousarray(out_i))


# revision 5
# speedup vs baseline: 1.0068x; 1.0068x over previous
"""Trainium2 Bass kernel for nn_ComplexMixture.

Reference:
  output_real[b,n,m] = sum_s w[b,s] * (r[b,s,n]*r[b,s,m] + i[b,s,n]*i[b,s,m])
  output_imag[b,n,m] = sum_s w[b,s] * (i[b,s,n]*r[b,s,m] - r[b,s,n]*i[b,s,m])

Shapes: B=32, S=128, N=256. w is uniform [0,1) so sqrt(w) is real.

out_r is symmetric and out_i is antisymmetric, so the device only computes
  P = out_r + out_i
and the host recovers out_r = (P + P^T)/2, out_i = (P - P^T)/2.
The host pre-scales the inputs: Yr = sqrt(w)[:,None]*r, Yi = sqrt(w)[:,None]*i.
With U = Yr - Yi, V = Yr + Yi:
  P[n,m] = sum_s Yr[s,n]*U[s,m] + Yi[s,n]*V[s,m]
i.e. per 128-row output chunk c:  P_c = Yr_c.T @ U + Yi_c.T @ V  (PSUM accumulation).

All device data is fp16: halves DMA bytes vs fp32 and runs the PE at
1 cycle/row instead of 4 (PSUM accumulation stays fp32; end-to-end rel
err ~1e-3 vs the 2e-2 gate).

Data-parallel over B across 8 cores, 4 batches/core.

Per-core schedule (engine queues are FIFO; assignments below keep every
dependency chain's consumer on an otherwise-idle queue):
  sync:   dma-in b0, dma-in b2+b3, dma-out b0, b2, b3-half1
  scalar: dma-in b1, copy b1, dma-out b1, copy b3-half2, dma-out b3-half2
  vector: U=Yr-Yi for b0..b3, copy b0, b2, b3-half1 (PSUM fp32 -> fp16)
  gpsimd: junk memset (PE warmup data), V=Yr+Yi for b0..b3  (no SWDGE
          DMAs at all -- avoids the ~2.3us SWDGE dge_drain at exit)
  tensor: N_WARMUP f32r warmup matmuls on junk (ramps the PE p-state to
          full clock during the input-DMA dead time), then 16 fp16
          matmuls (4 per batch).
"""

import os

import numpy as np

import concourse.bass as bass
import concourse.mybir as mybir
import concourse.tile as tile
from concourse import bacc
from concourse.bass_utils import run_bass_kernel_spmd

B, S, N = 32, 128, 256
NCORES = 8
BPC = B // NCORES  # batches per core
XCOL = 2 * N * BPC  # 2048 fp16 columns

F32 = mybir.dt.float32
F16 = mybir.dt.float16
N_WARMUP = int(os.environ.get("CM_WARMUP", "15"))

LAST_RESULTS = None  # stashed BassKernelResults for test harness introspection


def build_nc() -> bass.Bass:
    nc = bacc.Bacc(num_swdge_queues=1)
    xin = nc.dram_tensor("xpack", [S, XCOL], F16, kind="ExternalInput")
    out = nc.dram_tensor("out_all", [BPC, 128, 2, N], F16, kind="ExternalOutput")

    with tile.TileContext(nc) as tc:
        with (
            tc.tile_pool(name="io", bufs=1) as io_pool,
            tc.tile_pool(name="yp", bufs=BPC) as y_pool,
            tc.tile_pool(name="op", bufs=BPC) as out_pool,
            tc.tile_pool(name="ps", bufs=BPC, space="PSUM") as ps_pool,
            tc.tile_pool(name="wu", bufs=1, space="PSUM") as wu_pool,
        ):
            # PE warmup scratch: small memset so the first warmup matmul can
            # start as early as possible after the tile-entry barrier.
            if N_WARMUP:
                junk = io_pool.tile([S, 128], F32, tag="junk", name="junk")
                nc.gpsimd.memset(junk, 1.0)
                junk_r = junk.bitcast(mybir.dt.float32r)
                wups = wu_pool.tile([128, 128], F32, tag="wu", name="wups")
                for k in range(N_WARMUP):
                    nc.tensor.matmul(
                        wups, lhsT=junk_r, rhs=junk_r,
                        start=True, stop=True, skip_group_check=True,
                    )

            X_all = io_pool.tile([S, XCOL], F16, tag="X", name="X_all")
            BW = 2 * N  # columns per batch
            # All input on the two HWDGE rings: b0 then b2+b3 on SP (sync),
            # b1 on ACT (scalar).
            nc.sync.dma_start(out=X_all[:, 0:BW], in_=xin[:, 0:BW])
            nc.scalar.dma_start(out=X_all[:, BW : 2 * BW], in_=xin[:, BW : 2 * BW])
            nc.sync.dma_start(out=X_all[:, 2 * BW : 4 * BW], in_=xin[:, 2 * BW : 4 * BW])

            def xparts(b):
                X = X_all[:, b * BW : (b + 1) * BW]
                return X[:, 0:N], X[:, N : 2 * N]

            # Phase 1: UV transforms. GpSimd's tensor_tensor is ~2.6x slower
            # than DVE (750ns vs 292ns for [128,256] fp16), so vector does 7
            # of the 8 ops; gpsimd only does b3's V-add, which it can start
            # as soon as b3's data lands while vector is still mid-queue.
            UVs = []
            for b in range(BPC):
                Yr, Yi = xparts(b)
                UV = y_pool.tile([S, 2 * N], F16, tag="UV", name=f"UV{b}")
                nc.vector.tensor_sub(UV[:, 0:N], Yr, Yi)
                if b == BPC - 1:
                    nc.gpsimd.tensor_add(UV[:, N : 2 * N], Yr, Yi)
                else:
                    nc.vector.tensor_add(UV[:, N : 2 * N], Yr, Yi)
                UVs.append(UV)

            # Phase 2: matmuls (tensor engine).
            pss = []
            for b in range(BPC):
                Yr, Yi = xparts(b)
                UV = UVs[b]
                ps = ps_pool.tile([128, 2 * N], F32, tag="ps", name=f"ps{b}")
                for c in range(2):
                    csl = slice(c * 128, c * 128 + 128)
                    osl = slice(c * N, (c + 1) * N)
                    nc.tensor.matmul(ps[:, osl], lhsT=Yr[:, csl], rhs=UV[:, 0:N], start=True, stop=False)
                    nc.tensor.matmul(ps[:, osl], lhsT=Yi[:, csl], rhs=UV[:, N : 2 * N], start=False, stop=True)
                pss.append(ps)

            # Phase 3: PSUM -> SBUF fp16 downcast copies + output DMAs.
            Os = [out_pool.tile([128, 2 * N], F16, tag="O", name=f"O{b}") for b in range(BPC)]
            dsts = [out[b].rearrange("p c m -> p (c m)") for b in range(BPC)]

            # b0: vector copy, sync kick
            nc.vector.tensor_copy(Os[0], pss[0])
            nc.sync.dma_start(out=dsts[0], in_=Os[0])
            # b1: scalar copy, scalar kick
            nc.scalar.copy(out=Os[1], in_=pss[1])
            nc.scalar.dma_start(out=dsts[1], in_=Os[1])
            # b2: vector copy, sync kick
            nc.vector.tensor_copy(Os[2], pss[2])
            nc.sync.dma_start(out=dsts[2], in_=Os[2])
            # b3: split halves across vector+scalar, kicks on sync+scalar
            nc.vector.tensor_copy(Os[3][:, 0:N], pss[3][:, 0:N])
            nc.sync.dma_start(out=out[3][:, 0, :], in_=Os[3][:, 0:N])
            nc.scalar.copy(out=Os[3][:, N : 2 * N], in_=pss[3][:, N : 2 * N])
            nc.scalar.dma_start(out=out[3][:, 1, :], in_=Os[3][:, N : 2 * N])
    nc.compile()
    return nc


def kernel(**inputs: np.ndarray):
    global LAST_RESULTS
    r = np.asarray(inputs["input_real"], dtype=np.float32)
    i = np.asarray(inputs["input_imag"], dtype=np.float32)
    w = np.ascontiguousarray(np.asarray(inputs["weight"], dtype=np.float32))
    assert r.shape == (B, S, N) and i.shape == (B, S, N) and w.shape == (B, S)

    # [B, 2, S, N] -> per-core [S, (b t n)] batch-major blocks, fp16
    sws = np.sqrt(w)  # [B, S]
    xin = (np.stack([r, i], axis=1) * sws[:, None, :, None]).astype(np.float16)

    in_maps = []
    for c in range(NCORES):
        sl = slice(c * BPC, (c + 1) * BPC)
        xpack = np.transpose(xin[sl], (2, 0, 1, 3)).reshape(S, XCOL)
        in_maps.append({"xpack": np.ascontiguousarray(xpack)})

    nc = build_nc()
    res = run_bass_kernel_spmd(nc, in_maps, core_ids=list(range(NCORES)))
    LAST_RESULTS = res

    out_all = np.concatenate(
        [np.asarray(res.results[c]["out_all"]) for c in range(NCORES)], axis=0
    )  # [B, 128, 2, N] fp16; P[b, c*128+p, m] = out_all[b, p, c, m]
    P = np.transpose(out_all.astype(np.float32), (0, 2, 1, 3)).reshape(B, N, N)
    Pt = np.transpose(P, (0, 2, 1))
    out_r = (P + Pt) * np.float32(0.5)
    out_i = (P - Pt) * np.float32(0.5)
    return (np.ascontiguousarray(out_r), np.ascontiguousarray(out_i))


# revision 6
# speedup vs baseline: 1.0278x; 1.0209x over previous
"""Trainium2 Bass kernel for nn_ComplexMixture.

Reference:
  output_real[b,n,m] = sum_s w[b,s] * (r[b,s,n]*r[b,s,m] + i[b,s,n]*i[b,s,m])
  output_imag[b,n,m] = sum_s w[b,s] * (i[b,s,n]*r[b,s,m] - r[b,s,n]*i[b,s,m])

Shapes: B=32, S=128, N=256. w is uniform [0,1) so sqrt(w) is real.

out_r is symmetric and out_i is antisymmetric, so the device only computes
  P = out_r + out_i
and the host recovers out_r = (P + P^T)/2, out_i = (P - P^T)/2.
The host pre-scales the inputs: Yr = sqrt(w)[:,None]*r, Yi = sqrt(w)[:,None]*i.
With U = Yr - Yi, V = Yr + Yi:
  P[n,m] = sum_s Yr[s,n]*U[s,m] + Yi[s,n]*V[s,m]
i.e. per 128-row output chunk c:  P_c = Yr_c.T @ U + Yi_c.T @ V  (PSUM accumulation).

All device data is fp16: halves DMA bytes vs fp32 and runs the PE at
1 cycle/row instead of 4 (PSUM accumulation stays fp32; end-to-end rel
err ~1e-3 vs the 2e-2 gate).

Data-parallel over B across 8 cores, 4 batches/core.

Key measured facts this schedule is built around (per-instruction NTFF):
  - PE HAM clock gate releases (K=4/8 -> 8/8) only after ~3.4us of
    *fp16/bf16* PE busy time; f32r warmups never released it. So the
    warmup matmuls are fp16 and sized to ~3.4us, then the 16 real
    matmuls run warm (~107ns each instead of 213).
  - A dma_start costs ~0.6us of issuing-engine time (128 descriptors)
    + ~0.7us first-byte + ~0.85us completion receipt. Fewer, bigger
    DMAs win when the data is ready together -> output is a single
    [128, 2048] SBUF tile DMA'd as two [128,1024] pair-kicks on the
    two HWDGE rings (out DRAM layout is partition-major so each kick
    is 2KB/partition contiguous, still 128 descriptors).
  - GpSimd tensor_tensor is ~2.6x slower than DVE; it only does b3's
    V-add (started early, off vector's queue). No SWDGE DMAs at all
    (avoids the ~2.3us dge_drain at exit).
"""

import os

import numpy as np

import concourse.bass as bass
import concourse.mybir as mybir
import concourse.tile as tile
from concourse import bacc
from concourse.bass_utils import run_bass_kernel_spmd

B, S, N = 32, 128, 256
NCORES = 8
BPC = B // NCORES  # batches per core
XCOL = 2 * N * BPC  # 2048 fp16 columns

F32 = mybir.dt.float32
F16 = mybir.dt.float16
N_WARMUP = int(os.environ.get("CM_WARMUP", "8"))

LAST_RESULTS = None  # stashed BassKernelResults for test harness introspection


def build_nc() -> bass.Bass:
    nc = bacc.Bacc(num_swdge_queues=1)
    xin = nc.dram_tensor("xpack", [S, XCOL], F16, kind="ExternalInput")
    # partition-major output: out[p, b, c, m]; per partition all 4 batches
    # are contiguous so pair-kicks stay one descriptor per partition.
    out = nc.dram_tensor("out_all", [128, BPC, 2, N], F16, kind="ExternalOutput")

    with tile.TileContext(nc) as tc:
        with (
            tc.tile_pool(name="io", bufs=1) as io_pool,
            tc.tile_pool(name="yp", bufs=BPC) as y_pool,
            tc.tile_pool(name="op", bufs=1) as out_pool,
            tc.tile_pool(name="ps", bufs=BPC, space="PSUM") as ps_pool,
            tc.tile_pool(name="wu", bufs=1, space="PSUM") as wu_pool,
        ):
            # fp16 PE warmup: HAM wants ~3.4us of fp16 busy time before it
            # un-throttles; each cold ap-512 fp16 matmul is ~427ns.
            if N_WARMUP:
                junk = io_pool.tile([S, 512], F16, tag="junk", name="junk")
                nc.gpsimd.memset(junk, 1.0)
                wups = wu_pool.tile([128, 512], F32, tag="wu", name="wups")
                for k in range(N_WARMUP):
                    nc.tensor.matmul(
                        wups, lhsT=junk[:, 0:128], rhs=junk,
                        start=True, stop=True, skip_group_check=True,
                    )

            X_all = io_pool.tile([S, XCOL], F16, tag="X", name="X_all")
            BW = 2 * N  # columns per batch
            # All input on the two HWDGE rings: b0 then b2+b3 on SP (sync),
            # b1 on ACT (scalar; shares the ring with the one-time ACT table
            # load, so only the latest-needed single batch goes there).
            nc.sync.dma_start(out=X_all[:, 0:BW], in_=xin[:, 0:BW])
            nc.scalar.dma_start(out=X_all[:, BW : 2 * BW], in_=xin[:, BW : 2 * BW])
            nc.sync.dma_start(out=X_all[:, 2 * BW : 4 * BW], in_=xin[:, 2 * BW : 4 * BW])

            def xparts(b):
                X = X_all[:, b * BW : (b + 1) * BW]
                return X[:, 0:N], X[:, N : 2 * N]

            # Phase 1: UV transforms. Vector does 7 of 8 (292ns each);
            # gpsimd only b3's V-add (750ns, but off the critical queue).
            UVs = []
            for b in range(BPC):
                Yr, Yi = xparts(b)
                UV = y_pool.tile([S, 2 * N], F16, tag="UV", name=f"UV{b}")
                nc.vector.tensor_sub(UV[:, 0:N], Yr, Yi)
                if b == BPC - 1:
                    nc.gpsimd.tensor_add(UV[:, N : 2 * N], Yr, Yi)
                else:
                    nc.vector.tensor_add(UV[:, N : 2 * N], Yr, Yi)
                UVs.append(UV)

            # Phase 2: matmuls (tensor engine).
            pss = []
            for b in range(BPC):
                Yr, Yi = xparts(b)
                UV = UVs[b]
                ps = ps_pool.tile([128, 2 * N], F32, tag="ps", name=f"ps{b}")
                for c in range(2):
                    csl = slice(c * 128, c * 128 + 128)
                    osl = slice(c * N, (c + 1) * N)
                    nc.tensor.matmul(ps[:, osl], lhsT=Yr[:, csl], rhs=UV[:, 0:N], start=True, stop=False)
                    nc.tensor.matmul(ps[:, osl], lhsT=Yi[:, csl], rhs=UV[:, N : 2 * N], start=False, stop=True)
                pss.append(ps)

            # Phase 3: PSUM -> SBUF fp16 downcast copies into one [128,2048]
            # tile, then two [128,1024] pair DMAs on the two HWDGE rings.
            O = out_pool.tile([128, BPC * 2 * N], F16, tag="O", name="O")

            def ocols(b, half=None):
                base = b * 2 * N
                if half is None:
                    return O[:, base : base + 2 * N]
                return O[:, base + half * N : base + (half + 1) * N]

            # b0: scalar (free earliest), b1: vector (after its UV queue),
            # b2: scalar, b3: split halves vector+scalar.
            nc.scalar.copy(out=ocols(0), in_=pss[0])
            nc.vector.tensor_copy(ocols(1), pss[1])
            nc.scalar.copy(out=ocols(2), in_=pss[2])
            nc.vector.tensor_copy(ocols(3, 0), pss[3][:, 0:N])
            nc.scalar.copy(out=ocols(3, 1), in_=pss[3][:, N : 2 * N])

            dst = out.rearrange("p b c m -> p (b c m)")
            half = BPC * N  # 1024 columns
            nc.sync.dma_start(out=dst[:, 0:half], in_=O[:, 0:half])
            nc.scalar.dma_start(out=dst[:, half : 2 * half], in_=O[:, half : 2 * half])
    nc.compile()
    return nc


def kernel(**inputs: np.ndarray):
    global LAST_RESULTS
    r = np.asarray(inputs["input_real"], dtype=np.float32)
    i = np.asarray(inputs["input_imag"], dtype=np.float32)
    w = np.ascontiguousarray(np.asarray(inputs["weight"], dtype=np.float32))
    assert r.shape == (B, S, N) and i.shape == (B, S, N) and w.shape == (B, S)

    # [B, 2, S, N] -> per-core [S, (b t n)] batch-major blocks, fp16
    sws = np.sqrt(w)  # [B, S]
    xin = (np.stack([r, i], axis=1) * sws[:, None, :, None]).astype(np.float16)

    in_maps = []
    for c in range(NCORES):
        sl = slice(c * BPC, (c + 1) * BPC)
        xpack = np.transpose(xin[sl], (2, 0, 1, 3)).reshape(S, XCOL)
        in_maps.append({"xpack": np.ascontiguousarray(xpack)})

    nc = build_nc()
    res = run_bass_kernel_spmd(nc, in_maps, core_ids=list(range(NCORES)))
    LAST_RESULTS = res

    # out[p, b, c, m] per core; P[cb*BPC+b, c*128+p, m] = out[p, b, c, m]
    out_all = np.stack(
        [np.asarray(res.results[c]["out_all"]) for c in range(NCORES)], axis=0
    ).astype(np.float32)  # [NCORES, 128, BPC, 2, N]
    P = np.transpose(out_all, (0, 2, 3, 1, 4)).reshape(B, N, N)
    Pt = np.transpose(P, (0, 2, 1))
    out_r = (P + Pt) * np.float32(0.5)
    out_i = (P - Pt) * np.float32(0.5)
    return (np.ascontiguousarray(out_r), np.ascontiguousarray(out_i))


# revision 7
# speedup vs baseline: 1.0657x; 1.0369x over previous
"""Trainium2 Bass kernel for nn_ComplexMixture.

Reference:
  output_real[b,n,m] = sum_s w[b,s] * (r[b,s,n]*r[b,s,m] + i[b,s,n]*i[b,s,m])
  output_imag[b,n,m] = sum_s w[b,s] * (i[b,s,n]*r[b,s,m] - r[b,s,n]*i[b,s,m])

Shapes: B=32, S=128, N=256. w is uniform [0,1) so sqrt(w) is real.

out_r is symmetric and out_i is antisymmetric, so the device only computes
  P = out_r + out_i
and the host recovers out_r = (P + P^T)/2, out_i = (P - P^T)/2.
The host pre-scales the inputs: Yr = sqrt(w)[:,None]*r, Yi = sqrt(w)[:,None]*i.
With U = Yr - Yi, V = Yr + Yi:
  P[n,m] = sum_s Yr[s,n]*U[s,m] + Yi[s,n]*V[s,m]
i.e. per 128-row output chunk c:  P_c = Yr_c.T @ U + Yi_c.T @ V  (PSUM accumulation).

All device data is fp16 (DMA bytes halved vs fp32, PE at 1 cycle/row,
PSUM accumulation fp32; end-to-end rel err ~1e-3 vs the 2e-2 gate).
Data-parallel over B across 8 cores, 4 batches/core.

Measured facts this schedule is built around (per-instruction NTFF):
  - PE HAM releases its clock gate (K=4/8 -> 8/8, 213ns -> 109ns per
    256-wide fp16 matmul) after ~3.4us of *fp16* PE busy time; f32r
    warmups never release it. Warmups are fp16 and the release may land
    mid-way through the real matmuls (first few run cold regardless).
  - dma_start ~0.6us issue + ~0.7us first-byte + ~0.85us receipt.
  - Copies are chunk-level ([128,256] ~0.42us) so PSUM drains while the
    PE is still on later batches; output leaves in three kicks.
  - GpSimd tensor_tensor is 2.6x slower than DVE but has an empty
    queue: it takes b2/b3's V-adds. No SWDGE DMAs (avoids the ~2.3us
    dge_drain at exit).
"""

import os

import numpy as np

import concourse.bass as bass
import concourse.mybir as mybir
import concourse.tile as tile
from concourse import bacc
from concourse.bass_utils import run_bass_kernel_spmd

B, S, N = 32, 128, 256
NCORES = 8
BPC = B // NCORES  # batches per core
XCOL = 2 * N * BPC  # 2048 fp16 columns

F32 = mybir.dt.float32
F16 = mybir.dt.float16
N_WARMUP = int(os.environ.get("CM_WARMUP", "6"))
# uint32 bit pattern of two fp16 1.0s; exactly representable in fp32.
FP16_ONES_U32 = float(0x3C003C00)

LAST_RESULTS = None  # stashed BassKernelResults for test harness introspection


def build_nc() -> bass.Bass:
    nc = bacc.Bacc(num_swdge_queues=1)
    xin = nc.dram_tensor("xpack", [S, XCOL], F16, kind="ExternalInput")
    # partition-major output: out[p, b, c, m]; per partition all 4 batches
    # contiguous so multi-batch kicks stay one descriptor per partition.
    out = nc.dram_tensor("out_all", [128, BPC, 2, N], F16, kind="ExternalOutput")

    with tile.TileContext(nc) as tc:
        with (
            tc.tile_pool(name="io", bufs=1) as io_pool,
            tc.tile_pool(name="yp", bufs=BPC) as y_pool,
            tc.tile_pool(name="op", bufs=1) as out_pool,
            tc.tile_pool(name="ps", bufs=BPC, space="PSUM") as ps_pool,
            tc.tile_pool(name="wu", bufs=1, space="PSUM") as wu_pool,
        ):
            # fp16 PE warmup. Memset through a uint32 view halves the fill
            # time so the first warmup matmul issues as early as possible.
            if N_WARMUP:
                junk = io_pool.tile([S, 512], F16, tag="junk", name="junk")
                nc.gpsimd.memset(junk.bitcast(mybir.dt.uint32), FP16_ONES_U32)
                wups = wu_pool.tile([128, 512], F32, tag="wu", name="wups")
                for k in range(N_WARMUP):
                    nc.tensor.matmul(
                        wups, lhsT=junk[:, 0:128], rhs=junk,
                        start=True, stop=True, skip_group_check=True,
                    )

            X_all = io_pool.tile([S, XCOL], F16, tag="X", name="X_all")
            BW = 2 * N  # columns per batch
            # All input on the two HWDGE rings: b0 then b2+b3 on SP (sync),
            # b1 on ACT (scalar; that ring also carries the one-time ACT
            # table load, so only one batch goes there).
            nc.sync.dma_start(out=X_all[:, 0:BW], in_=xin[:, 0:BW])
            nc.scalar.dma_start(out=X_all[:, BW : 2 * BW], in_=xin[:, BW : 2 * BW])
            nc.sync.dma_start(out=X_all[:, 2 * BW : 4 * BW], in_=xin[:, 2 * BW : 4 * BW])

            def xparts(b):
                X = X_all[:, b * BW : (b + 1) * BW]
                return X[:, 0:N], X[:, N : 2 * N]

            # Phase 1: UV transforms. Vector: subs + adds for b0/b1;
            # gpsimd (slower, idle queue): adds for b2/b3.
            UVs = []
            for b in range(BPC):
                Yr, Yi = xparts(b)
                UV = y_pool.tile([S, 2 * N], F16, tag="UV", name=f"UV{b}")
                nc.vector.tensor_sub(UV[:, 0:N], Yr, Yi)
                if b >= 2:
                    nc.gpsimd.tensor_add(UV[:, N : 2 * N], Yr, Yi)
                else:
                    nc.vector.tensor_add(UV[:, N : 2 * N], Yr, Yi)
                UVs.append(UV)

            # Phase 2: matmuls (tensor engine), chunk pairs per batch.
            pss = []
            for b in range(BPC):
                Yr, Yi = xparts(b)
                UV = UVs[b]
                ps = ps_pool.tile([128, 2 * N], F32, tag="ps", name=f"ps{b}")
                for c in range(2):
                    csl = slice(c * 128, c * 128 + 128)
                    osl = slice(c * N, (c + 1) * N)
                    nc.tensor.matmul(ps[:, osl], lhsT=Yr[:, csl], rhs=UV[:, 0:N], start=True, stop=False)
                    nc.tensor.matmul(ps[:, osl], lhsT=Yi[:, csl], rhs=UV[:, N : 2 * N], start=False, stop=True)
                pss.append(ps)

            # Phase 3: chunk-level PSUM -> SBUF fp16 copies (start as soon
            # as each accumulate pair stops), then three output kicks.
            O = out_pool.tile([128, BPC * 2 * N], F16, tag="O", name="O")

            def ocol(b, c):
                base = b * 2 * N + c * N
                return O[:, base : base + N]

            # chunk (b,c): c=0 chunks on scalar, c=1 chunks on vector
            # (vector is busy with UV until ~b1's chunks are ready).
            for b in range(BPC):
                nc.scalar.copy(out=ocol(b, 0), in_=pss[b][:, 0:N])
                nc.vector.tensor_copy(ocol(b, 1), pss[b][:, N : 2 * N])

            dst = out.rearrange("p b c m -> p (b c m)")
            CW = 2 * N
            # b0+b1 on sync, b2 on scalar, b3 on sync.
            nc.sync.dma_start(out=dst[:, 0 : 2 * CW], in_=O[:, 0 : 2 * CW])
            nc.scalar.dma_start(out=dst[:, 2 * CW : 3 * CW], in_=O[:, 2 * CW : 3 * CW])
            nc.sync.dma_start(out=dst[:, 3 * CW : 4 * CW], in_=O[:, 3 * CW : 4 * CW])
    nc.compile()
    return nc


def kernel(**inputs: np.ndarray):
    global LAST_RESULTS
    r = np.asarray(inputs["input_real"], dtype=np.float32)
    i = np.asarray(inputs["input_imag"], dtype=np.float32)
    w = np.ascontiguousarray(np.asarray(inputs["weight"], dtype=np.float32))
    assert r.shape == (B, S, N) and i.shape == (B, S, N) and w.shape == (B, S)

    # [B, 2, S, N] -> per-core [S, (b t n)] batch-major blocks, fp16
    sws = np.sqrt(w)  # [B, S]
    xin = (np.stack([r, i], axis=1) * sws[:, None, :, None]).astype(np.float16)

    in_maps = []
    for c in range(NCORES):
        sl = slice(c * BPC, (c + 1) * BPC)
        xpack = np.transpose(xin[sl], (2, 0, 1, 3)).reshape(S, XCOL)
        in_maps.append({"xpack": np.ascontiguousarray(xpack)})

    nc = build_nc()
    res = run_bass_kernel_spmd(nc, in_maps, core_ids=list(range(NCORES)))
    LAST_RESULTS = res

    # out[p, b, c, m] per core; P[cb*BPC+b, c*128+p, m] = out[p, b, c, m]
    out_all = np.stack(
        [np.asarray(res.results[c]["out_all"]) for c in range(NCORES)], axis=0
    ).astype(np.float32)  # [NCORES, 128, BPC, 2, N]
    P = np.transpose(out_all, (0, 2, 3, 1, 4)).reshape(B, N, N)
    Pt = np.transpose(P, (0, 2, 1))
    out_r = (P + Pt) * np.float32(0.5)
    out_i = (P - Pt) * np.float32(0.5)
    return (np.ascontiguousarray(out_r), np.ascontiguousarray(out_i))
